# revision 6
# baseline (speedup 1.0000x reference)
"""MinGRU Trainium2 kernel (v3: bf16 datapath, 4+1 conv split, combined DMA).

Reference computation (per batch b):
    c = depthwise_conv1d(x, conv_w, taps=5, pad=2)        # [D, L]
    h = h_w @ c                                           # [O, L]
    g = concat([-1000, +1000], g_w @ c)                   # [O, L]
    a = sigmoid(-g); v = (1-a) * h
    out[l] = a[l] * out[l-1] + v[l]     (linear scan along L)

Strategy: pure data-parallel over B (8 batches -> 8 NeuronCores).
Per core, everything streams in l-chunks of 512:
  - all matmul inputs bf16 (x, conv diag weights, h/g weights); PSUM f32
  - conv: taps {0,1,3,4} as diagonal-matmuls on TensorE accumulating in PSUM;
    the center tap is fused into the PSUM->SBUF move as a single DVE
    scalar_tensor_tensor  c = x2*w2 + cpsum  (saves a matmul per d-tile AND
    frees ScalarE of the copy)
  - h/g 1x1-conv matmuls on TensorE (bf16 rhs = c)
  - a = sigmoid(-(g+bias)) on ScalarE; bias carries the +/-1000 polarized
    rows, so scan rows 0/1 equal 0 / h automatically (within error budget)
  - h weights negated on the host so v = (1-a)*h = (a-1)*(-h) is one fused
    DVE scalar_tensor_tensor reading -h straight from PSUM
  - scan via tensor_tensor_scan (DVE), bf16 in/out with f32 internal state
  - x / weights / outputs move as one large DMA per tensor per chunk
  - dummy matmuls chained on the tail tiles keep the Tensor engine "active"
    through the drain phase so the HAM does not duty-throttle the tail
  - out stored bf16, converted to f32 on the host
"""

import numpy as np
import ml_dtypes

import concourse.bass as bass
import concourse.mybir as mybir
from concourse import bacc
from concourse.tile import TileContext
from concourse.bass_utils import run_bass_kernel_spmd

F32 = mybir.dt.float32
BF16 = mybir.dt.bfloat16
AF = mybir.ActivationFunctionType
OP = mybir.AluOpType

B, D, O, L = 8, 512, 512, 4096
P = 128
CH = 512                 # l-chunk width (one PSUM bank)
CHW = CH + 4             # x chunk width incl. 2-col halo each side
NCH = L // CH            # 8
NDT = D // P             # 4 d-tiles
NOT = O // P             # 4 o-tiles
NTAPS = 5
N_CORES = 8


def build_program():
    nc = bacc.Bacc()

    x = nc.declare_dram_parameter("x", [D, L], BF16, isOutput=False)
    hwTn = nc.declare_dram_parameter("hwTn", [D, O], BF16, isOutput=False)
    gwT = nc.declare_dram_parameter("gwT", [D, O], BF16, isOutput=False)
    cwdiag = nc.declare_dram_parameter("cwdiag", [D, (NTAPS - 1) * P], BF16,
                                       isOutput=False)
    cwmid = nc.declare_dram_parameter("cwmid", [D, 1], F32, isOutput=False)
    gbn = nc.declare_dram_parameter("gbn", [O, 1], F32, isOutput=False)
    zpad = nc.declare_dram_parameter("zpad", [P, 2], BF16, isOutput=False)
    out = nc.declare_dram_parameter("out", [O, L], BF16, isOutput=True)
    dbg = nc.declare_dram_parameter("dbg", [2, 2], F32, isOutput=True)

    with TileContext(nc) as tc:
        with (
            tc.tile_pool(name="weights", bufs=1) as wpool,
            tc.tile_pool(name="xin", bufs=4) as xpool,
            tc.tile_pool(name="csb", bufs=10) as cpool,
            tc.tile_pool(name="actout", bufs=12) as apool,
            tc.tile_pool(name="vtiles", bufs=12) as vpool,
            tc.tile_pool(name="outt", bufs=3) as opool,
            tc.tile_pool(name="cps", bufs=4, space="PSUM") as cps_pool,
            tc.tile_pool(name="hps", bufs=2, space="PSUM") as hps_pool,
            tc.tile_pool(name="gps", bufs=2, space="PSUM") as gps_pool,
        ):
            # Scalar HWDGE queue: warmup tile, conv weights (one combined
            # trigger each), then the per-chunk x loads. Sync queue: stores.
            # GpSimd SWDGE: the two big h/g weight matrices.
            warm_sb = wpool.tile([P, 2], BF16, tag="warm")
            nc.scalar.dma_start(out=warm_sb, in_=zpad[:, :])
            cw4 = wpool.tile([P, NDT * (NTAPS - 1) * P], BF16, tag="cw4")
            nc.scalar.dma_start(
                out=cw4.rearrange("p (q c) -> p q c", c=(NTAPS - 1) * P),
                in_=cwdiag[:, :].rearrange("(q p) c -> p q c", p=P))
            cwm4 = wpool.tile([P, NDT], F32, tag="cwm4")
            nc.scalar.dma_start(
                out=cwm4.rearrange("p (q c) -> p q c", c=1),
                in_=cwmid[:, :].rearrange("(q p) c -> p q c", p=P))
            gbn4 = wpool.tile([P, NOT], F32, tag="gbn4")
            nc.scalar.dma_start(
                out=gbn4.rearrange("p (q c) -> p q c", c=1),
                in_=gbn[:, :].rearrange("(q p) c -> p q c", p=P))
            gw4 = wpool.tile([P, NDT * O], BF16, tag="gw4")
            nc.gpsimd.dma_start(
                out=gw4.rearrange("p (q o) -> p q o", o=O),
                in_=gwT[:, :].rearrange("(q p) o -> p q o", p=P))
            hw4 = wpool.tile([P, NDT * O], BF16, tag="hw4")
            nc.gpsimd.dma_start(
                out=hw4.rearrange("p (q o) -> p q o", o=O),
                in_=hwTn[:, :].rearrange("(q p) o -> p q o", p=P))

            c_sb = [None] * NCH          # [chunk] -> list of 4 SBUF c tiles
            prev_out = [None] * NCH      # [chunk] -> combined out tile
            tail_tiles = []              # late tiles for the HAM keep-alive

            def emit_conv(i):
                lo = i * CH
                # one combined x DMA for all 4 d-tiles (halo included)
                xt = xpool.tile([P, NDT * CHW], BF16, tag="xt")
                xv = xt.rearrange("p (q c) -> p q c", c=CHW)
                if i == 0:
                    for dt in range(NDT):
                        nc.scalar.dma_start(out=xt[:, dt * CHW:dt * CHW + 2],
                                            in_=zpad[:, :])
                    nc.scalar.dma_start(
                        out=xv[:, :, 2:CHW],
                        in_=x[:, 0:CH + 2].rearrange("(q p) l -> p q l", p=P))
                elif i == NCH - 1:
                    for dt in range(NDT):
                        nc.scalar.dma_start(
                            out=xt[:, dt * CHW + CH + 2:dt * CHW + CHW],
                            in_=zpad[:, :])
                    nc.scalar.dma_start(
                        out=xv[:, :, 0:CH + 2],
                        in_=x[:, lo - 2:lo + CH].rearrange(
                            "(q p) l -> p q l", p=P))
                else:
                    nc.scalar.dma_start(
                        out=xv,
                        in_=x[:, lo - 2:lo + CH + 2].rearrange(
                            "(q p) l -> p q l", p=P))
                tiles = []
                for dt in range(NDT):
                    cp = cps_pool.tile([P, CH], F32, tag="cps")
                    # taps 1,3 first: their windows never touch the zpad halo
                    # columns, so the first matmuls don't wait on the tiny
                    # edge DMAs
                    for j, k in enumerate((1, 3, 0, 4)):
                        kk = k if k < 2 else k - 1   # diag block index
                        nc.tensor.matmul(
                            cp,
                            lhsT=cw4[:, (dt * (NTAPS - 1) + kk) * P:
                                     (dt * (NTAPS - 1) + kk + 1) * P],
                            rhs=xt[:, dt * CHW + k:dt * CHW + k + CH],
                            start=(j == 0), stop=(j == 3),
                        )
                    # center tap fused with the PSUM->SBUF move:
                    # c = x2*w2 + cpsum
                    ct = cpool.tile([P, CH], BF16, tag="ct")
                    nc.vector.scalar_tensor_tensor(
                        ct, xt[:, dt * CHW + 2:dt * CHW + 2 + CH],
                        cwm4[:, dt:dt + 1], cp, OP.mult, OP.add)
                    tiles.append(ct)
                c_sb[i] = tiles

            def emit_rest(i):
                lo = i * CH
                ott = opool.tile([P, NOT * CH], BF16, tag="ott")
                for ot in range(NOT):
                    # g before h: the sigmoid chain (ACT) only needs g, so it
                    # starts while the h matmuls are still streaming
                    gp = gps_pool.tile([P, CH], F32, tag="gps")
                    for dt in range(NDT):
                        nc.tensor.matmul(
                            gp,
                            lhsT=gw4[:, dt * O + ot * P:dt * O + (ot + 1) * P],
                            rhs=c_sb[i][dt],
                            start=(dt == 0), stop=(dt == NDT - 1),
                        )
                    hp = hps_pool.tile([P, CH], F32, tag="hps")
                    for dt in range(NDT):
                        nc.tensor.matmul(
                            hp,
                            lhsT=hw4[:, dt * O + ot * P:dt * O + (ot + 1) * P],
                            rhs=c_sb[i][dt],
                            start=(dt == 0), stop=(dt == NDT - 1),
                        )
                    # a = sigmoid(-(g + bias)); v = (a-1)*(-h)
                    at = apool.tile([P, CH], BF16, tag="at")
                    nc.scalar.activation(at, gp, AF.Sigmoid,
                                         bias=gbn4[:, ot:ot + 1], scale=-1.0)
                    vt = vpool.tile([P, CH], BF16, tag="vt")
                    nc.vector.scalar_tensor_tensor(vt, at, 1.0, hp,
                                                   OP.subtract, OP.mult)
                    init = (0.0 if i == 0 else
                            prev_out[i - 1][:, ot * CH + CH - 1:ot * CH + CH])
                    nc.vector.tensor_tensor_scan(
                        ott[:, ot * CH:(ot + 1) * CH], at, vt, init,
                        OP.mult, OP.add)
                    if i >= NCH - 2:
                        tail_tiles.append((at, vt))
                nc.sync.dma_start(
                    out=out[:, lo:lo + CH].rearrange("(q p) l -> p q l", p=P),
                    in_=ott.rearrange("p (q l) -> p q l", l=CH))
                prev_out[i] = ott

            # PE warm-up: dummy matmuls on the zero tile during the initial
            # DMA wait trip the HAM clock gate to full speed before real work
            # arrives; the result (zeros) lands in an out region that the
            # chunk-0 store overwrites anyway.
            wps = cps_pool.tile([P, CH], F32, tag="cps", name="warmps")
            for _ in range(28):
                nc.tensor.matmul(wps[0:2, 0:2], lhsT=warm_sb, rhs=warm_sb,
                                 start=True, stop=True)
            wout = wpool.tile([2, 2], BF16, tag="warmout")
            nc.vector.tensor_copy(wout, wps[0:2, 0:2])
            nc.gpsimd.dma_start(out=out[2:4, 0:2], in_=wout)

            # chunk pairs, software-pipelined one pair ahead
            emit_conv(0)
            emit_conv(1)
            for p in range(1, NCH // 2):
                emit_conv(2 * p)
                emit_conv(2 * p + 1)
                emit_rest(2 * p - 2)
                emit_rest(2 * p - 1)
            emit_rest(NCH - 2)
            emit_rest(NCH - 1)

            # HAM keep-alive: dummy matmuls chained on the tail's a/v/out
            # tiles keep the Tensor engine active through the drain phase so
            # the activity monitor does not halve the duty cycle under the
            # final scans.
            kps = gps_pool.tile([P, CH], F32, tag="gps", name="keepps")
            for (at, vt) in tail_tiles:
                nc.tensor.matmul(kps[0:2, :], lhsT=warm_sb, rhs=at,
                                 start=True, stop=True)
                nc.tensor.matmul(kps[0:2, :], lhsT=warm_sb, rhs=vt,
                                 start=True, stop=True)
            for ot in range(NOT):
                for i in (NCH - 2, NCH - 1):
                    nc.tensor.matmul(
                        kps[0:2, :], lhsT=warm_sb,
                        rhs=prev_out[i][:, ot * CH:(ot + 1) * CH],
                        start=True, stop=True)
            kout = wpool.tile([2, 2], F32, tag="keepout")
            nc.scalar.copy(kout, kps[0:2, 0:2])
            nc.sync.dma_start(out=dbg[:, :], in_=kout)

    nc.finalize()
    return nc


_PROGRAM = None


def _get_program():
    global _PROGRAM
    if _PROGRAM is None:
        _PROGRAM = build_program()
    return _PROGRAM


def prepare_in_maps(x, conv_w, h_w, g_w):
    BF = ml_dtypes.bfloat16
    x = np.asarray(x, dtype=np.float32)
    conv_w = np.asarray(conv_w, dtype=np.float32)
    h_w = np.asarray(h_w, dtype=np.float32)
    g_w = np.asarray(g_w, dtype=np.float32)

    xb = np.ascontiguousarray(x).astype(BF)                       # [B, D, L]
    hwTn = np.ascontiguousarray(-h_w[:, :, 0].T).astype(BF)       # [D, O]
    gw_pad = np.zeros((O, D), np.float32)
    gw_pad[2:, :] = g_w[:, :, 0]
    gwT = np.ascontiguousarray(gw_pad.T).astype(BF)               # [D, O]

    # diagonal matrices for taps 0,1,3,4 per d-tile: [D, 4*128]; the center
    # tap rides the DVE merge as a per-partition scalar
    cwdiag = np.zeros((D, (NTAPS - 1) * P), np.float32)
    for dt in range(NDT):
        for j, k in enumerate((0, 1, 3, 4)):
            blk = cwdiag[dt * P:(dt + 1) * P, j * P:(j + 1) * P]
            np.fill_diagonal(blk, conv_w[dt * P:(dt + 1) * P, 0, k])
    cwdiag = cwdiag.astype(BF)
    cwmid = np.ascontiguousarray(conv_w[:, :, 2]).astype(np.float32)  # [D,1]

    gbp = np.zeros((O, 1), np.float32)
    gbp[0, 0], gbp[1, 0] = -1000.0, 1000.0
    gbn = np.ascontiguousarray(-gbp)

    zpad = np.zeros((P, 2), BF)
    return [
        {"x": xb[b], "hwTn": hwTn, "gwT": gwT, "cwdiag": cwdiag,
         "cwmid": cwmid, "gbn": gbn, "zpad": zpad}
        for b in range(B)
    ]


def kernel(x, conv_w, h_w, g_w):
    in_maps = prepare_in_maps(x, conv_w, h_w, g_w)
    nc = _get_program()
    res = run_bass_kernel_spmd(nc, in_maps, list(range(N_CORES))).results
    return np.stack([np.asarray(res[b]["out"]).astype(np.float32)
                     for b in range(B)], axis=0)


# revision 7
# speedup vs baseline: 1.0282x; 1.0282x over previous
"""MinGRU Trainium2 kernel (v4: bf16 datapath, 4+1 conv split, p-major
weight layouts, HAM keep-alive tail).

Reference computation (per batch b):
    c = depthwise_conv1d(x, conv_w, taps=5, pad=2)        # [D, L]
    h = h_w @ c                                           # [O, L]
    g = concat([-1000, +1000], g_w @ c)                   # [O, L]
    a = sigmoid(-g); v = (1-a) * h
    out[l] = a[l] * out[l-1] + v[l]     (linear scan along L)

Strategy: pure data-parallel over B (8 batches -> 8 NeuronCores).
Per core, everything streams in l-chunks of 512:
  - all matmul inputs bf16 (x, conv diag weights, h/g weights); PSUM f32
  - conv: taps {0,1,3,4} as diagonal-matmuls on TensorE accumulating in PSUM;
    the center tap is fused into the PSUM->SBUF move as a single DVE
    scalar_tensor_tensor  c = x2*w2 + cpsum
  - weight tensors are uploaded pre-shuffled to p-major [128, ...] layouts so
    every weight DMA is one trigger with large contiguous per-partition rows
  - h/g 1x1-conv matmuls on TensorE (bf16 rhs = c)
  - a = sigmoid(-(g+bias)) on ScalarE; bias carries the +/-1000 polarized
    rows, so scan rows 0/1 equal 0 / h automatically (within error budget)
  - h weights negated on the host so v = (1-a)*h = (a-1)*(-h) is one fused
    DVE scalar_tensor_tensor reading -h straight from PSUM
  - scan via tensor_tensor_scan (DVE), bf16 in/out with f32 internal state
  - dummy matmuls chained on the tail tiles keep the Tensor engine "active"
    through the drain phase so the HAM does not duty-throttle the tail
  - out stored bf16 (one combined DMA per chunk), converted to f32 on host
"""

import numpy as np
import ml_dtypes

import concourse.bass as bass
import concourse.mybir as mybir
from concourse import bacc
from concourse.tile import TileContext
from concourse.bass_utils import run_bass_kernel_spmd

F32 = mybir.dt.float32
BF16 = mybir.dt.bfloat16
AF = mybir.ActivationFunctionType
OP = mybir.AluOpType

B, D, O, L = 8, 512, 512, 4096
P = 128
CH = 512                 # l-chunk width (one PSUM bank)
CHW = CH + 4             # x chunk width incl. 2-col halo each side
NCH = L // CH            # 8
NDT = D // P             # 4 d-tiles
NOT = O // P             # 4 o-tiles
NTAPS = 5
N_CORES = 8


def build_program():
    nc = bacc.Bacc()

    x = nc.declare_dram_parameter("x", [D, L], BF16, isOutput=False)
    # p-major pre-shuffled weight layouts (see prepare_in_maps)
    hw4d = nc.declare_dram_parameter("hw4d", [P, NDT * O], BF16, isOutput=False)
    gw4d = nc.declare_dram_parameter("gw4d", [P, NDT * O], BF16, isOutput=False)
    cw4d = nc.declare_dram_parameter("cw4d", [P, NDT * (NTAPS - 1) * P], BF16,
                                     isOutput=False)
    cwm4d = nc.declare_dram_parameter("cwm4d", [P, NDT], F32, isOutput=False)
    gbn4d = nc.declare_dram_parameter("gbn4d", [P, NOT], F32, isOutput=False)
    zpad = nc.declare_dram_parameter("zpad", [P, 2], BF16, isOutput=False)
    out = nc.declare_dram_parameter("out", [O, L], BF16, isOutput=True)
    dbg = nc.declare_dram_parameter("dbg", [2, 2], F32, isOutput=True)

    with TileContext(nc) as tc:
        with (
            tc.tile_pool(name="weights", bufs=1) as wpool,
            tc.tile_pool(name="xin", bufs=4) as xpool,
            tc.tile_pool(name="csb", bufs=10) as cpool,
            tc.tile_pool(name="actout", bufs=12) as apool,
            tc.tile_pool(name="vtiles", bufs=12) as vpool,
            tc.tile_pool(name="outt", bufs=3) as opool,
            tc.tile_pool(name="cps", bufs=3, space="PSUM") as cps_pool,
            tc.tile_pool(name="hps", bufs=3, space="PSUM") as hps_pool,
            tc.tile_pool(name="gps", bufs=2, space="PSUM") as gps_pool,
        ):
            # Sync queue leads with the warm-up tile and the first two x
            # chunks (it has nothing else queued); Scalar queue carries the
            # conv weights and the remaining x chunks; the two big h/g weight
            # matrices ride the GpSimd SWDGE queue.
            warm_sb = wpool.tile([P, 2], BF16, tag="warm")
            nc.sync.dma_start(out=warm_sb, in_=zpad[:, :])
            cw4 = wpool.tile([P, NDT * (NTAPS - 1) * P], BF16, tag="cw4")
            nc.scalar.dma_start(out=cw4, in_=cw4d[:, :])
            cwm4 = wpool.tile([P, NDT], F32, tag="cwm4")
            nc.scalar.dma_start(out=cwm4, in_=cwm4d[:, :])
            gbn4 = wpool.tile([P, NOT], F32, tag="gbn4")
            nc.scalar.dma_start(out=gbn4, in_=gbn4d[:, :])
            gw4 = wpool.tile([P, NDT * O], BF16, tag="gw4")
            nc.gpsimd.dma_start(out=gw4, in_=gw4d[:, :])
            hw4 = wpool.tile([P, NDT * O], BF16, tag="hw4")
            nc.gpsimd.dma_start(out=hw4, in_=hw4d[:, :])

            c_sb = [None] * NCH          # [chunk] -> list of 4 SBUF c tiles
            prev_out = [None] * NCH      # [chunk] -> combined out tile
            tail_tiles = []              # late tiles for the HAM keep-alive

            def emit_conv(i):
                lo = i * CH
                xq = nc.sync if i < 2 else nc.scalar
                # one combined x DMA for all 4 d-tiles (halo included)
                xt = xpool.tile([P, NDT * CHW], BF16, tag="xt")
                xv = xt.rearrange("p (q c) -> p q c", c=CHW)
                if i == 0:
                    for dt in range(NDT):
                        xq.dma_start(out=xt[:, dt * CHW:dt * CHW + 2],
                                     in_=zpad[:, :])
                    xq.dma_start(
                        out=xv[:, :, 2:CHW],
                        in_=x[:, 0:CH + 2].rearrange("(q p) l -> p q l", p=P))
                elif i == NCH - 1:
                    for dt in range(NDT):
                        xq.dma_start(
                            out=xt[:, dt * CHW + CH + 2:dt * CHW + CHW],
                            in_=zpad[:, :])
                    xq.dma_start(
                        out=xv[:, :, 0:CH + 2],
                        in_=x[:, lo - 2:lo + CH].rearrange(
                            "(q p) l -> p q l", p=P))
                else:
                    xq.dma_start(
                        out=xv,
                        in_=x[:, lo - 2:lo + CH + 2].rearrange(
                            "(q p) l -> p q l", p=P))
                tiles = []
                for dt in range(NDT):
                    cp = cps_pool.tile([P, CH], F32, tag="cps")
                    # taps 1,3 first: their windows never touch the zpad halo
                    # columns, so the first matmuls don't wait on the tiny
                    # edge DMAs
                    for j, k in enumerate((1, 3, 0, 4)):
                        kk = k if k < 2 else k - 1   # diag block index
                        nc.tensor.matmul(
                            cp,
                            lhsT=cw4[:, (dt * (NTAPS - 1) + kk) * P:
                                     (dt * (NTAPS - 1) + kk + 1) * P],
                            rhs=xt[:, dt * CHW + k:dt * CHW + k + CH],
                            start=(j == 0), stop=(j == 3),
                        )
                    # center tap fused with the PSUM->SBUF move:
                    # c = x2*w2 + cpsum
                    ct = cpool.tile([P, CH], BF16, tag="ct")
                    nc.vector.scalar_tensor_tensor(
                        ct, xt[:, dt * CHW + 2:dt * CHW + 2 + CH],
                        cwm4[:, dt:dt + 1], cp, OP.mult, OP.add)
                    tiles.append(ct)
                c_sb[i] = tiles

            def emit_rest(i):
                lo = i * CH
                ott = opool.tile([P, NOT * CH], BF16, tag="ott")
                for ot in range(NOT):
                    # g before h: the sigmoid chain (ACT) only needs g, so it
                    # starts while the h matmuls are still streaming
                    gp = gps_pool.tile([P, CH], F32, tag="gps")
                    for dt in range(NDT):
                        nc.tensor.matmul(
                            gp,
                            lhsT=gw4[:, dt * O + ot * P:dt * O + (ot + 1) * P],
                            rhs=c_sb[i][dt],
                            start=(dt == 0), stop=(dt == NDT - 1),
                        )
                    hp = hps_pool.tile([P, CH], F32, tag="hps")
                    for dt in range(NDT):
                        nc.tensor.matmul(
                            hp,
                            lhsT=hw4[:, dt * O + ot * P:dt * O + (ot + 1) * P],
                            rhs=c_sb[i][dt],
                            start=(dt == 0), stop=(dt == NDT - 1),
                        )
                    # a = sigmoid(-(g + bias)); v = (a-1)*(-h)
                    at = apool.tile([P, CH], BF16, tag="at")
                    nc.scalar.activation(at, gp, AF.Sigmoid,
                                         bias=gbn4[:, ot:ot + 1], scale=-1.0)
                    vt = vpool.tile([P, CH], BF16, tag="vt")
                    nc.vector.scalar_tensor_tensor(vt, at, 1.0, hp,
                                                   OP.subtract, OP.mult)
                    init = (0.0 if i == 0 else
                            prev_out[i - 1][:, ot * CH + CH - 1:ot * CH + CH])
                    nc.vector.tensor_tensor_scan(
                        ott[:, ot * CH:(ot + 1) * CH], at, vt, init,
                        OP.mult, OP.add)
                    if i >= NCH - 2:
                        tail_tiles.append((at, vt))
                nc.sync.dma_start(
                    out=out[:, lo:lo + CH].rearrange("(q p) l -> p q l", p=P),
                    in_=ott.rearrange("p (q l) -> p q l", l=CH))
                prev_out[i] = ott

            # PE warm-up: dummy matmuls on the zero tile during the initial
            # DMA wait trip the HAM clock gate to full speed before real work
            # arrives; the result (zeros) lands in an out region that the
            # chunk-0 store overwrites anyway.
            wps = cps_pool.tile([P, CH], F32, tag="cps", name="warmps")
            for _ in range(28):
                nc.tensor.matmul(wps[0:2, 0:2], lhsT=warm_sb, rhs=warm_sb,
                                 start=True, stop=True)
            wout = wpool.tile([2, 2], BF16, tag="warmout")
            nc.scalar.copy(wout, wps[0:2, 0:2])
            nc.gpsimd.dma_start(out=out[2:4, 0:2], in_=wout)

            # chunk pairs, software-pipelined one pair ahead
            emit_conv(0)
            emit_conv(1)
            for p in range(1, NCH // 2):
                emit_conv(2 * p)
                emit_conv(2 * p + 1)
                emit_rest(2 * p - 2)
                emit_rest(2 * p - 1)
            emit_rest(NCH - 2)
            emit_rest(NCH - 1)

            # HAM keep-alive: dummy matmuls chained on the tail's a/v/out
            # tiles keep the Tensor engine active through the drain phase so
            # the activity monitor does not halve the duty cycle under the
            # final scans.
            kps = gps_pool.tile([P, CH], F32, tag="gps", name="keepps")
            for (at, vt) in tail_tiles:
                nc.tensor.matmul(kps[0:2, :], lhsT=warm_sb, rhs=at,
                                 start=True, stop=True)
                nc.tensor.matmul(kps[0:2, :], lhsT=warm_sb, rhs=vt,
                                 start=True, stop=True)
            for ot in range(NOT):
                for i in (NCH - 2, NCH - 1):
                    nc.tensor.matmul(
                        kps[0:2, :], lhsT=warm_sb,
                        rhs=prev_out[i][:, ot * CH:(ot + 1) * CH],
                        start=True, stop=True)
            kout = wpool.tile([2, 2], F32, tag="keepout")
            nc.scalar.copy(kout, kps[0:2, 0:2])
            nc.sync.dma_start(out=dbg[:, :], in_=kout)

    nc.finalize()
    return nc


_PROGRAM = None


def _get_program():
    global _PROGRAM
    if _PROGRAM is None:
        _PROGRAM = build_program()
    return _PROGRAM


def _pmajor(a):
    """[NDT*P, X] -> [P, NDT*X] p-major shuffle: out[p, q*X+x] = a[q*P+p, x]"""
    q = a.shape[0] // P
    return np.ascontiguousarray(
        a.reshape(q, P, -1).transpose(1, 0, 2).reshape(P, -1))


def prepare_in_maps(x, conv_w, h_w, g_w):
    BF = ml_dtypes.bfloat16
    x = np.asarray(x, dtype=np.float32)
    conv_w = np.asarray(conv_w, dtype=np.float32)
    h_w = np.asarray(h_w, dtype=np.float32)
    g_w = np.asarray(g_w, dtype=np.float32)

    xb = np.ascontiguousarray(x).astype(BF)                       # [B, D, L]
    hw4d = _pmajor(-h_w[:, :, 0].T).astype(BF)                    # [P, 4*O]
    gw_pad = np.zeros((O, D), np.float32)
    gw_pad[2:, :] = g_w[:, :, 0]
    gw4d = _pmajor(gw_pad.T).astype(BF)                           # [P, 4*O]

    # diagonal matrices for taps 0,1,3,4 per d-tile: p-major [P, 4*4*128];
    # the center tap rides the DVE merge as a per-partition scalar
    cwdiag = np.zeros((D, (NTAPS - 1) * P), np.float32)
    for dt in range(NDT):
        for j, k in enumerate((0, 1, 3, 4)):
            blk = cwdiag[dt * P:(dt + 1) * P, j * P:(j + 1) * P]
            np.fill_diagonal(blk, conv_w[dt * P:(dt + 1) * P, 0, k])
    cw4d = _pmajor(cwdiag).astype(BF)
    cwm4d = _pmajor(conv_w[:, :, 2]).astype(np.float32)           # [P, 4]

    gbn = np.zeros((O, 1), np.float32)
    gbn[0, 0], gbn[1, 0] = 1000.0, -1000.0     # -(-1000), -(+1000)
    gbn4d = _pmajor(gbn).astype(np.float32)                       # [P, 4]

    zpad = np.zeros((P, 2), BF)
    return [
        {"x": xb[b], "hw4d": hw4d, "gw4d": gw4d, "cw4d": cw4d,
         "cwm4d": cwm4d, "gbn4d": gbn4d, "zpad": zpad}
        for b in range(B)
    ]


def kernel(x, conv_w, h_w, g_w):
    in_maps = prepare_in_maps(x, conv_w, h_w, g_w)
    nc = _get_program()
    res = run_bass_kernel_spmd(nc, in_maps, list(range(N_CORES))).results
    return np.stack([np.asarray(res[b]["out"]).astype(np.float32)
                     for b in range(B)], axis=0)
